# revision 4
# baseline (speedup 1.0000x reference)
"""DeepSeekExpert (fp8-quantized MLP expert) Trainium2 Bass kernel.

Computes, matching reference.py numerics:
    xq, xs = per_token_cast_to_fp8(x)          # per (token, 128-block) e4m3fn
    w1q, w1s = per_block_cast_to_fp8(w1)       # per 128x128 block
    o0  = dequant(xq,xs) @ dequant(w1q,w1s).T  # [S, F] bf16
    act = silu(o0)
    out = (act * o0) @ w2.T                    # [S, H] bf16
(w3 / o1 are dead in the reference and skipped.)

Sharding: tokens (rows of x) split across 8 cores; each core holds full
w1/w2 and processes S/8 tokens end to end.

Per-core pipeline:
  phase X : quantize+dequantize x tiles in natural layout (fp8 grid is
            reproduced exactly: scale = amax/448 via a true divide, RNE
            cast to Trainium fp8e4 of value/2, dequant by 2*scale), then
            SBUF->SBUF DMA-transpose into resident xdT [h, s].
  phase W1: quantize+dequantize w1 (block amax via free-dim abs_max
            reduce + gpsimd partition_all_reduce), write w1d to a DRAM
            scratch.
  phase C : per f-tile, DMA-transpose w1d tiles back as lhsT and run
            PSUM-accumulated matmul chains against xdT; silu epilogue
            into resident hT [f, s].
  phase D : per 512-wide output chunk, DMA-transpose w2 into rhs tiles
            and run matmul chains with hT as lhsT; copy PSUM->SBUF bf16
            and DMA out.
"""

import os
import sys
import types

os.environ.setdefault("JAX_COMPILATION_CACHE_DIR", "/tmp/jax_neff_cache")
os.environ.setdefault("JAX_PERSISTENT_CACHE_MIN_COMPILE_TIME_SECS", "1")
os.environ.setdefault("JAX_PERSISTENT_CACHE_MIN_ENTRY_SIZE_BYTES", "0")

import numpy as np


def build_program(NS, H, F, num_devices=8):
    """Trace + compile the per-core Bass program.

    NS: tokens per core.  H: hidden (x/w1 inner, out width).  F: ff dim.
    """
    import concourse.bacc as bacc
    import concourse.tile as tile
    from concourse import mybir
    from concourse import bass_isa

    BF16 = mybir.dt.bfloat16
    F32 = mybir.dt.float32
    FP8 = mybir.dt.float8e4
    MUL = mybir.AluOpType.mult
    X_AX = mybir.AxisListType.X

    P = 128
    KB = H // P          # h-blocks
    FB = F // P          # f-tiles
    ST = NS // P         # s-tiles per core
    SH = NS // 512       # 512-wide s-chunks
    HH = H // 512        # 512-wide output chunks
    XCH = 2              # x processed in half-width chunks
    XKB = KB // XCH
    WCH = max(d for d in range(1, 9) if KB % d == 0)  # w1 chunking (8 at full size)
    WKB = KB // WCH
    assert KB % XCH == 0 and NS % 512 == 0 and H % 512 == 0

    nc = bacc.Bacc(
        "TRN2", target_bir_lowering=False, debug=False, num_devices=num_devices
    )
    x_d = nc.dram_tensor("x", [NS, H], BF16, kind="ExternalInput")
    w1_d = nc.dram_tensor("w1", [F, H], BF16, kind="ExternalInput")
    w2_d = nc.dram_tensor("w2", [H, F], BF16, kind="ExternalInput")
    out_d = nc.dram_tensor("out", [NS, H], BF16, kind="ExternalOutput")

    def bc(scale_ap, nkb):
        # [128, nkb] f32 -> [128, nkb, 128] with stride-0 inner broadcast
        return scale_ap.unsqueeze(2).broadcast_to([P, nkb, P])

    # Split 1/448 so that s = RN(amax*c_hi + amax*c_lo) is exactly
    # RN(amax/448): amax is bf16-valued (8-bit mantissa) so amax*c_hi is
    # exact, and m/7 binary expansions have no long same-bit runs, so the
    # final rounding always agrees with true division.
    _c = np.float64(1.0) / np.float64(448.0)
    _m, _e = np.frexp(_c)
    C448_HI = float(np.float32(np.ldexp(np.floor(np.ldexp(_m, 16)), int(_e) - 16)))
    C448_LO = float(np.float32(_c - np.float64(C448_HI)))

    def quant_scales(pool, amax, nkb, tagp):
        """amax [128, nkb] f32 (already abs-max) -> (rs, s2): rs = RNE(0.5/scale),
        s2 = 2*scale, with scale = RNE(clip(amax)/448) exactly as the reference."""
        nc.vector.tensor_scalar_max(amax[:], amax[:], 1e-4)
        s = pool.tile([P, nkb], F32, tag=tagp + "_s")
        nc.vector.tensor_scalar_mul(s[:], amax[:], C448_LO)
        nc.vector.scalar_tensor_tensor(
            s[:], amax[:], C448_HI, s[:],
            op0=mybir.AluOpType.mult, op1=mybir.AluOpType.add,
        )
        rs = pool.tile([P, nkb], F32, tag=tagp + "_rs")
        nc.vector.reciprocal(rs[:], s[:])
        nc.vector.tensor_scalar_mul(rs[:], rs[:], 0.5)
        s2 = pool.tile([P, nkb], F32, tag=tagp + "_s2")
        nc.vector.tensor_scalar_mul(s2[:], s[:], 2.0)
        return rs, s2

    with tile.TileContext(nc) as tc, tc.tile_pool(name="hT", bufs=1) as p_hT:
        hT = p_hT.tile([P, FB * NS], BF16)
        with (
            tc.tile_pool(name="xdT", bufs=1) as p_xdT,
            tc.tile_pool(name="scr", bufs=1, space="DRAM") as p_dram,
        ):
            xdT = p_xdT.tile([P, KB * NS], BF16)
            w1scr = p_dram.tile([F, H], BF16)

            # ---------------- phase X ----------------
            with (
                tc.tile_pool(name="xw", bufs=3) as p_xw,
                tc.tile_pool(name="xq", bufs=2) as p_xq,
                tc.tile_pool(name="xs", bufs=2) as p_xs,
            ):
                CW = XKB * P  # chunk width
                for st in range(ST):
                    chunks = []
                    amax = p_xs.tile([P, KB], F32, tag="amax")
                    for c in range(XCH):
                        xt = p_xw.tile([P, CW], BF16, tag="xt")
                        nc.sync.dma_start(
                            xt[:],
                            x_d.ap()[st * P:(st + 1) * P, c * CW:(c + 1) * CW],
                        )
                        xt3 = xt[:].rearrange("p (k b) -> p k b", b=P)
                        nc.vector.tensor_reduce(
                            amax[:, c * XKB:(c + 1) * XKB], xt3, axis=X_AX,
                            op=mybir.AluOpType.max, apply_absolute_value=True,
                        )
                        chunks.append(xt3)
                    rs, s2 = quant_scales(p_xs, amax, KB, "x")
                    for c in range(XCH):
                        ksl = slice(c * XKB, (c + 1) * XKB)
                        q8 = p_xq.tile([P, CW], FP8, tag="q8")
                        q83 = q8[:].rearrange("p (k b) -> p k b", b=P)
                        nc.vector.tensor_tensor(
                            q83, chunks[c], bc(rs[:, ksl], XKB), op=MUL
                        )
                        xd = p_xq.tile([P, CW], BF16, tag="xd")
                        xd3 = xd[:].rearrange("p (k b) -> p k b", b=P)
                        nc.vector.tensor_tensor(
                            xd3, q83, bc(s2[:, ksl], XKB), op=MUL
                        )
                        for k in range(XKB):
                            kb = c * XKB + k
                            nc.sync.dma_start_transpose(
                                xdT[:, kb * NS + st * P: kb * NS + (st + 1) * P],
                                xd[:, k * P:(k + 1) * P],
                            )

            # ---------------- phases W1 + C (interleaved per f-tile) -----
            with (
                tc.tile_pool(name="wt", bufs=WCH + 2) as p_wt,
                tc.tile_pool(name="wq", bufs=2) as p_wq,
                tc.tile_pool(name="wsc", bufs=2) as p_wsc,
                tc.tile_pool(name="cw", bufs=2) as p_cw,
                tc.tile_pool(name="cs", bufs=3) as p_cs,
                tc.tile_pool(name="psA", bufs=4, space="PSUM") as p_psA,
            ):
                WW = WKB * P
                for fb in range(FB):
                    fsl = slice(fb * P, (fb + 1) * P)
                    # quantize w1[fb] in natural layout
                    wrow = p_wsc.tile([P, KB], F32, tag="wrow")
                    wts = []
                    for c in range(WCH):
                        wt = p_wt.tile([P, WW], BF16, tag="wt")
                        nc.sync.dma_start(
                            wt[:], w1_d.ap()[fsl, c * WW:(c + 1) * WW]
                        )
                        wt3 = wt[:].rearrange("p (k b) -> p k b", b=P)
                        nc.vector.tensor_reduce(
                            wrow[:, c * WKB:(c + 1) * WKB], wt3, axis=X_AX,
                            op=mybir.AluOpType.max, apply_absolute_value=True,
                        )
                        wts.append(wt3)
                    wam = p_wsc.tile([P, KB], F32, tag="wam")
                    nc.gpsimd.partition_all_reduce(
                        wam[:], wrow[:], channels=P,
                        reduce_op=bass_isa.ReduceOp.absmax,
                    )
                    wrs, ws2 = quant_scales(p_wsc, wam, KB, "w")
                    for c in range(WCH):
                        ksl = slice(c * WKB, (c + 1) * WKB)
                        q8w = p_wq.tile([P, WW], FP8, tag="q8w")
                        q8w3 = q8w[:].rearrange("p (k b) -> p k b", b=P)
                        nc.vector.tensor_tensor(
                            q8w3, wts[c], bc(wrs[:, ksl], WKB), op=MUL
                        )
                        wdq = p_wq.tile([P, WW], BF16, tag="wdq")
                        wdq3 = wdq[:].rearrange("p (k b) -> p k b", b=P)
                        nc.vector.tensor_tensor(
                            wdq3, q8w3, bc(ws2[:, ksl], WKB), op=MUL
                        )
                        nc.sync.dma_start(
                            w1scr[fsl, c * WW:(c + 1) * WW], wdq[:]
                        )
                    # gemm1 chains for this f-tile
                    w1dT = p_cw.tile([P, KB * P], BF16, tag="w1dT")
                    for kb in range(KB):
                        nc.sync.dma_start_transpose(
                            w1dT[:, kb * P:(kb + 1) * P],
                            w1scr[fsl, kb * P:(kb + 1) * P],
                        )
                    for sh in range(SH):
                        ps = p_psA.tile([P, 512], F32, tag="ps")
                        for kb in range(KB):
                            nc.tensor.matmul(
                                ps[:],
                                w1dT[:, kb * P:(kb + 1) * P],
                                xdT[:, kb * NS + sh * 512: kb * NS + sh * 512 + 512],
                                start=(kb == 0), stop=(kb == KB - 1),
                            )
                        o0b = p_cs.tile([P, 512], BF16, tag="o0b")
                        nc.vector.tensor_copy(o0b[:], ps[:])
                        sg = p_cs.tile([P, 512], BF16, tag="sg")
                        nc.scalar.activation(
                            sg[:], o0b[:], mybir.ActivationFunctionType.Sigmoid
                        )
                        act = p_cs.tile([P, 512], BF16, tag="act")
                        nc.vector.tensor_mul(act[:], o0b[:], sg[:])
                        nc.vector.tensor_mul(
                            hT[:, fb * NS + sh * 512: fb * NS + sh * 512 + 512],
                            act[:], o0b[:],
                        )

        # ---------------- phase D ----------------
        # (xdT + scratch pools released above; hT persists)
        with (
            tc.tile_pool(name="dw", bufs=2) as p_dw,
            tc.tile_pool(name="do", bufs=4) as p_do,
            tc.tile_pool(name="psB", bufs=4, space="PSUM") as p_psB,
        ):
            for hh in range(HH):
                w2T = p_dw.tile([P, FB * 512], BF16, tag="w2T")
                for fb in range(FB):
                    nc.sync.dma_start_transpose(
                        w2T[:, fb * 512:(fb + 1) * 512],
                        w2_d.ap()[hh * 512:(hh + 1) * 512, fb * P:(fb + 1) * P],
                    )
                for st in range(ST):
                    ps2 = p_psB.tile([P, 512], F32, tag="ps2")
                    for fb in range(FB):
                        nc.tensor.matmul(
                            ps2[:],
                            hT[:, fb * NS + st * P: fb * NS + (st + 1) * P],
                            w2T[:, fb * 512:(fb + 1) * 512],
                            start=(fb == 0), stop=(fb == FB - 1),
                        )
                    ob = p_do.tile([P, 512], BF16, tag="ob")
                    nc.vector.tensor_copy(ob[:], ps2[:])
                    nc.sync.dma_start(
                        out_d.ap()[st * P:(st + 1) * P, hh * 512:(hh + 1) * 512],
                        ob[:],
                    )

    nc.compile()
    return nc


_PROG_CACHE = {}


def _get_program(NS, H, F, num_devices=8):
    key = (NS, H, F, num_devices)
    if key not in _PROG_CACHE:
        _PROG_CACHE[key] = build_program(NS, H, F, num_devices)
    return _PROG_CACHE[key]


NCORES = 8


def kernel(x, w1, w2, w3=None, **_ignored):
    """Full-input entry point: shards tokens across 8 NeuronCores."""
    from concourse.bass_utils import run_bass_kernel_spmd

    x = np.asarray(x)
    w1 = np.asarray(w1)
    w2 = np.asarray(w2)
    S, H = x.shape
    F = w1.shape[0]
    NS = S // NCORES
    nc = _get_program(NS, H, F, NCORES)
    in_maps = [
        {
            "x": np.ascontiguousarray(x[i * NS:(i + 1) * NS]),
            "w1": w1,
            "w2": w2,
        }
        for i in range(NCORES)
    ]
    res = run_bass_kernel_spmd(nc, in_maps, core_ids=list(range(NCORES)))
    return np.concatenate(
        [res.results[i]["out"] for i in range(NCORES)], axis=0
    )


# revision 12
# speedup vs baseline: 1.5560x; 1.5560x over previous
"""DeepSeekExpert (fp8-quantized MLP expert) Trainium2 Bass kernel.

Computes, matching reference.py numerics:
    xq, xs = per_token_cast_to_fp8(x)          # per (token, 128-block) e4m3fn
    w1q, w1s = per_block_cast_to_fp8(w1)       # per 128x128 block
    o0  = dequant(xq,xs) @ dequant(w1q,w1s).T  # [S, F] bf16
    act = silu(o0)
    out = (act * o0) @ w2.T                    # [S, H] bf16
(w3 / o1 are dead in the reference and skipped.)

Sharding: tokens (rows of x) split across 8 cores; each core holds full
w1/w2 and processes S/8 tokens end to end.

Per-core pipeline (v2 — few large DMA-transposes, two 512-token passes):
  phase X : quantize+dequantize x tiles in natural layout (fp8 grid is
            reproduced exactly: scale = RN(amax/448) via a split-constant
            multiply-add, RNE cast to Trainium fp8e4 of value/2, dequant
            by 2*scale), write xd to DRAM scratch.
  phase W1: quantize+dequantize w1 the same way (block amax via free-dim
            abs_max reduce + gpsimd partition_all_reduce), write w1d to
            DRAM scratch.
  passes p=0,1 (512 tokens each):
     refill xdT [h,s] via 56 big DMA-transposes of the xd scratch,
     then per group of 4 f-tiles: 56 big DMA-transposes of w1d scratch
     into w1dT, PSUM-accumulated matmul chains, silu epilogue into
     resident hT [f, s].
  phase D : per 1024-wide output superset: DMA-transpose w2 into rhs
            tiles, matmul chains with hT slices as lhsT, PSUM->SBUF bf16
            copy, DMA out.
"""

import os

os.environ.setdefault("JAX_COMPILATION_CACHE_DIR", "/tmp/jax_neff_cache")
os.environ.setdefault("JAX_PERSISTENT_CACHE_MIN_COMPILE_TIME_SECS", "1")
os.environ.setdefault("JAX_PERSISTENT_CACHE_MIN_ENTRY_SIZE_BYTES", "0")

import numpy as np

TR_RING = os.environ.get("TR_RING", "scalar")  # sync | scalar | alt


def build_program(NS, H, F, num_devices=8):
    """Trace + compile the per-core Bass program.

    NS: tokens per core.  H: hidden (x/w1 inner, out width).  F: ff dim.
    """
    import concourse.bacc as bacc
    import concourse.tile as tile
    from concourse import mybir
    from concourse import bass_isa

    BF16 = mybir.dt.bfloat16
    F32 = mybir.dt.float32
    FP8 = mybir.dt.float8e4
    MUL = mybir.AluOpType.mult
    X_AX = mybir.AxisListType.X

    P = 128
    KB = H // P          # h-blocks
    FB = F // P          # f-tiles
    ST = NS // P         # s-tiles per core
    NP = NS // 512       # 512-token passes
    FG = 4               # f-tiles per gemm1 group
    SC = 1024 if H % 1024 == 0 else 512   # phase-D output superset width
    NSC = H // SC
    # quantization processes rows in NCH chunks of QKB h-blocks each
    NCH = min(n for n in range(1, KB + 1) if KB % n == 0 and KB // n <= 14)
    QKB = KB // NCH
    assert NS % 512 == 0 and H % 512 == 0 and FB % FG == 0

    nc = bacc.Bacc(
        "TRN2", target_bir_lowering=False, debug=False, num_devices=num_devices
    )
    x_d = nc.dram_tensor("x", [NS, H], BF16, kind="ExternalInput")
    w1_d = nc.dram_tensor("w1", [F, H], BF16, kind="ExternalInput")
    w2_d = nc.dram_tensor("w2", [H, F], BF16, kind="ExternalInput")
    out_d = nc.dram_tensor("out", [NS, H], BF16, kind="ExternalOutput")
    xscr_d = nc.dram_tensor("xscr", [NS, H], BF16, kind="ExternalOutput")

    # alternate DMA-transpose calls across the two HWDGE rings
    _ring = [0]

    def tr_dma(out_ap, in_ap):
        eng = nc.sync if TR_RING in ("sync", "alt") and (
            TR_RING == "sync" or _ring[0] % 2 == 0) else nc.scalar
        _ring[0] += 1
        eng.dma_start_transpose(out_ap, in_ap)

    def bc(scale_ap, nkb):
        # [128, nkb] f32 -> [128, nkb, 128] with stride-0 inner broadcast
        return scale_ap.unsqueeze(2).broadcast_to([P, nkb, P])

    # Split 1/448 so that s = RN(amax*c_hi + amax*c_lo) is exactly
    # RN(amax/448): amax is bf16-valued (8-bit mantissa) so amax*c_hi is
    # exact, and m/7 binary expansions have no long same-bit runs, so the
    # final rounding always agrees with true division.
    _c = np.float64(1.0) / np.float64(448.0)
    _m, _e = np.frexp(_c)
    C448_HI = float(np.float32(np.ldexp(np.floor(np.ldexp(_m, 16)), int(_e) - 16)))
    C448_LO = float(np.float32(_c - np.float64(C448_HI)))

    def quant_scales(pool, amax, nkb, tagp):
        """amax [128, nkb] f32 (abs-max) -> (rs, s2): rs = RNE(0.5/scale),
        s2 = 2*scale, scale = RNE(clip(amax)/448) exactly as the reference."""
        nc.vector.tensor_scalar_max(amax[:], amax[:], 1e-4)
        s = pool.tile([P, nkb], F32, tag=tagp + "_s")
        nc.vector.tensor_scalar_mul(s[:], amax[:], C448_LO)
        nc.vector.scalar_tensor_tensor(
            s[:], amax[:], C448_HI, s[:],
            op0=MUL, op1=mybir.AluOpType.add,
        )
        rs = pool.tile([P, nkb], F32, tag=tagp + "_rs")
        nc.vector.reciprocal(rs[:], s[:])
        nc.vector.tensor_scalar_mul(rs[:], rs[:], 0.5)
        s2 = pool.tile([P, nkb], F32, tag=tagp + "_s2")
        nc.vector.tensor_scalar_mul(s2[:], s[:], 2.0)
        return rs, s2

    with tile.TileContext(nc) as tc, tc.tile_pool(name="hT", bufs=1) as p_hT:
        hT = p_hT.tile([P, FB * NS], BF16)
        with (
            tc.tile_pool(name="xdT", bufs=1) as p_xdT,
            tc.tile_pool(name="scr", bufs=1, space="DRAM") as p_dram,
            tc.tile_pool(name="qw", bufs=NCH + 2) as p_qw,
            tc.tile_pool(name="qq", bufs=2) as p_qq,
            tc.tile_pool(name="qsc", bufs=2) as p_qsc,
            tc.tile_pool(name="cw", bufs=1) as p_cw,
            tc.tile_pool(name="cs", bufs=2) as p_cs,
            tc.tile_pool(name="psA", bufs=4, space="PSUM") as p_psA,
        ):
            xdT = p_xdT.tile([P, KB * 512], BF16)
            w1dT = p_cw.tile([P, KB * 512], BF16)
            xscr = xscr_d
            w1scr = p_dram.tile([F, H], BF16)
            QW = QKB * P

            def quant_rows(src_ap, dst_scr, rsl, pare, tagp):
                """Quantize+dequantize one 128-row strip of src (natural
                layout) into the DRAM scratch, reproducing the reference
                fp8 grid.  pare=True adds the cross-partition block-max
                (weights); else scales are per-row (tokens)."""
                amax = p_qsc.tile([P, KB], F32, tag="amax")
                chunks = []
                for c in range(NCH):
                    qt = p_qw.tile([P, QW], BF16, tag="nt")
                    nc.sync.dma_start(
                        qt[:], src_ap[rsl, c * QW:(c + 1) * QW]
                    )
                    qt3 = qt[:].rearrange("p (k b) -> p k b", b=P)
                    nc.vector.tensor_reduce(
                        amax[:, c * QKB:(c + 1) * QKB], qt3, axis=X_AX,
                        op=mybir.AluOpType.max, apply_absolute_value=True,
                    )
                    chunks.append(qt3)
                if pare:
                    wam = p_qsc.tile([P, KB], F32, tag="wam")
                    nc.gpsimd.partition_all_reduce(
                        wam[:], amax[:], channels=P,
                        reduce_op=bass_isa.ReduceOp.absmax,
                    )
                    amax = wam
                rs, s2 = quant_scales(p_qsc, amax, KB, tagp)
                for c in range(NCH):
                    ksl = slice(c * QKB, (c + 1) * QKB)
                    q8 = p_qq.tile([P, QW], FP8, tag="q8")
                    q83 = q8[:].rearrange("p (k b) -> p k b", b=P)
                    nc.vector.tensor_tensor(
                        q83, chunks[c], bc(rs[:, ksl], QKB), op=MUL
                    )
                    dq = p_qq.tile([P, QW], BF16, tag="dq")
                    dq3 = dq[:].rearrange("p (k b) -> p k b", b=P)
                    nc.vector.tensor_tensor(
                        dq3, q83, bc(s2[:, ksl], QKB), op=MUL
                    )
                    nc.sync.dma_start(
                        dst_scr[rsl, c * QW:(c + 1) * QW], dq[:]
                    )

            # phase X: quantize x rows, spill xd to scratch
            for st in range(ST):
                quant_rows(
                    x_d.ap(), xscr.ap(), slice(st * P, (st + 1) * P), False, "x"
                )
            # phase W1: quantize w1 rows (per-128x128-block), spill w1d
            for fb in range(FB):
                quant_rows(
                    w1_d.ap(), w1scr, slice(fb * P, (fb + 1) * P), True, "w"
                )

            # ---------------- passes: gemm1 + silu -> hT ---------------
            for p in range(NP):
                ssl = slice(p * 512, (p + 1) * 512)
                for kb in range(KB):
                    tr_dma(
                        xdT[:, kb * 512:(kb + 1) * 512],
                        xscr.ap()[ssl, kb * P:(kb + 1) * P],
                    )
                for fg in range(FB // FG):
                    for kb in range(KB):
                        tr_dma(
                            w1dT[:, kb * 512:(kb + 1) * 512],
                            w1scr[fg * FG * P:(fg + 1) * FG * P,
                                  kb * P:(kb + 1) * P],
                        )
                    for g in range(FG):
                        fb = fg * FG + g
                        ps = p_psA.tile([P, 512], F32, tag="ps")
                        for kb in range(KB):
                            nc.tensor.matmul(
                                ps[:],
                                w1dT[:, kb * 512 + g * P: kb * 512 + (g + 1) * P],
                                xdT[:, kb * 512:(kb + 1) * 512],
                                start=(kb == 0), stop=(kb == KB - 1),
                            )
                        o0b = p_cs.tile([P, 512], BF16, tag="o0b")
                        nc.scalar.copy(o0b[:], ps[:])
                        sg = p_cs.tile([P, 512], BF16, tag="sg")
                        nc.scalar.activation(
                            sg[:], o0b[:], mybir.ActivationFunctionType.Sigmoid
                        )
                        act = p_cs.tile([P, 512], BF16, tag="act")
                        nc.vector.tensor_mul(act[:], o0b[:], sg[:])
                        nc.vector.tensor_mul(
                            hT[:, fb * NS + p * 512: fb * NS + (p + 1) * 512],
                            act[:], o0b[:],
                        )

        # ---------------- phase D ----------------
        # (xdT/w1dT/scratch/w-pools released above; hT persists)
        with (
            tc.tile_pool(name="dw", bufs=2) as p_dw,
            tc.tile_pool(name="do", bufs=4) as p_do,
            tc.tile_pool(name="psB", bufs=4, space="PSUM") as p_psB,
        ):
            for sc in range(NSC):
                w2T = p_dw.tile([P, FB * SC], BF16, tag="w2T")
                for fb in range(FB):
                    tr_dma(
                        w2T[:, fb * SC:(fb + 1) * SC],
                        w2_d.ap()[sc * SC:(sc + 1) * SC, fb * P:(fb + 1) * P],
                    )
                for hsub in range(SC // 512):
                    for st in range(ST):
                        ps2 = p_psB.tile([P, 512], F32, tag="ps2")
                        for fb in range(FB):
                            nc.tensor.matmul(
                                ps2[:],
                                hT[:, fb * NS + st * P: fb * NS + (st + 1) * P],
                                w2T[:, fb * SC + hsub * 512: fb * SC + (hsub + 1) * 512],
                                start=(fb == 0), stop=(fb == FB - 1),
                            )
                        ob = p_do.tile([P, 512], BF16, tag="ob")
                        if st % 2 == 0:
                            nc.vector.tensor_copy(ob[:], ps2[:])
                        else:
                            nc.scalar.copy(ob[:], ps2[:])
                        nc.sync.dma_start(
                            out_d.ap()[st * P:(st + 1) * P,
                                       sc * SC + hsub * 512: sc * SC + (hsub + 1) * 512],
                            ob[:],
                        )

    nc.compile()
    return nc


_PROG_CACHE = {}


def _get_program(NS, H, F, num_devices=8):
    key = (NS, H, F, num_devices)
    if key not in _PROG_CACHE:
        _PROG_CACHE[key] = build_program(NS, H, F, num_devices)
    return _PROG_CACHE[key]


NCORES = 8


def kernel(x, w1, w2, w3=None, **_ignored):
    """Full-input entry point: shards tokens across 8 NeuronCores."""
    from concourse.bass_utils import run_bass_kernel_spmd

    x = np.asarray(x)
    w1 = np.asarray(w1)
    w2 = np.asarray(w2)
    S, H = x.shape
    F = w1.shape[0]
    NS = S // NCORES
    nc = _get_program(NS, H, F, NCORES)
    in_maps = [
        {
            "x": np.ascontiguousarray(x[i * NS:(i + 1) * NS]),
            "w1": w1,
            "w2": w2,
        }
        for i in range(NCORES)
    ]
    res = run_bass_kernel_spmd(nc, in_maps, core_ids=list(range(NCORES)))
    return np.concatenate(
        [res.results[i]["out"] for i in range(NCORES)], axis=0
    )


# revision 14
# speedup vs baseline: 1.6004x; 1.0285x over previous
"""DeepSeekExpert (fp8-quantized MLP expert) Trainium2 Bass kernel.

Computes, matching reference.py numerics:
    xq, xs = per_token_cast_to_fp8(x)          # per (token, 128-block) e4m3fn
    w1q, w1s = per_block_cast_to_fp8(w1)       # per 128x128 block
    o0  = dequant(xq,xs) @ dequant(w1q,w1s).T  # [S, F] bf16
    act = silu(o0)
    out = (act * o0) @ w2.T                    # [S, H] bf16
(w3 / o1 are dead in the reference and skipped.)

Sharding: tokens (rows of x) split across 8 cores; each core holds full
w1/w2 and processes S/8 tokens end to end.

Per-core pipeline (v2 — few large DMA-transposes, two 512-token passes):
  phase X : quantize+dequantize x tiles in natural layout (fp8 grid is
            reproduced exactly: scale = RN(amax/448) via a split-constant
            multiply-add, RNE cast to Trainium fp8e4 of value/2, dequant
            by 2*scale), write xd to DRAM scratch.
  phase W1: quantize+dequantize w1 the same way (block amax via free-dim
            abs_max reduce + gpsimd partition_all_reduce), write w1d to
            DRAM scratch.
  passes p=0,1 (512 tokens each):
     refill xdT [h,s] via 56 big DMA-transposes of the xd scratch,
     then per group of 4 f-tiles: 56 big DMA-transposes of w1d scratch
     into w1dT, PSUM-accumulated matmul chains, silu epilogue into
     resident hT [f, s].
  phase D : per 1024-wide output superset: DMA-transpose w2 into rhs
            tiles, matmul chains with hT slices as lhsT, PSUM->SBUF bf16
            copy, DMA out.
"""

import os

os.environ.setdefault("JAX_COMPILATION_CACHE_DIR", "/tmp/jax_neff_cache")
os.environ.setdefault("JAX_PERSISTENT_CACHE_MIN_COMPILE_TIME_SECS", "1")
os.environ.setdefault("JAX_PERSISTENT_CACHE_MIN_ENTRY_SIZE_BYTES", "0")

import numpy as np

TR_RING = os.environ.get("TR_RING", "scalar")  # sync | scalar | alt


def build_program(NS, H, F, num_devices=8):
    """Trace + compile the per-core Bass program.

    NS: tokens per core.  H: hidden (x/w1 inner, out width).  F: ff dim.
    """
    import concourse.bacc as bacc
    import concourse.tile as tile
    from concourse import mybir
    from concourse import bass_isa

    BF16 = mybir.dt.bfloat16
    F32 = mybir.dt.float32
    FP8 = mybir.dt.float8e4
    MUL = mybir.AluOpType.mult
    X_AX = mybir.AxisListType.X

    P = 128
    KB = H // P          # h-blocks
    FB = F // P          # f-tiles
    ST = NS // P         # s-tiles per core
    NP = NS // 512       # 512-token passes
    FG = 4               # f-tiles per gemm1 group
    SC = 1024 if H % 1024 == 0 else 512   # phase-D output superset width
    NSC = H // SC
    # quantization processes rows in NCH chunks of QKB h-blocks each
    NCH = min(n for n in range(1, KB + 1) if KB % n == 0 and KB // n <= 14)
    QKB = KB // NCH
    assert NS % 512 == 0 and H % 512 == 0 and FB % FG == 0

    nc = bacc.Bacc(
        "TRN2", target_bir_lowering=False, debug=False, num_devices=num_devices
    )
    x_d = nc.dram_tensor("x", [NS, H], BF16, kind="ExternalInput")
    w1_d = nc.dram_tensor("w1", [F, H], BF16, kind="ExternalInput")
    w2_d = nc.dram_tensor("w2", [H, F], BF16, kind="ExternalInput")
    out_d = nc.dram_tensor("out", [NS, H], BF16, kind="ExternalOutput")
    xscr_d = nc.dram_tensor("xscr", [NS, H], BF16, kind="ExternalOutput")

    # alternate DMA-transpose calls across the two HWDGE rings
    _ring = [0]

    def tr_dma(out_ap, in_ap):
        eng = nc.sync if TR_RING in ("sync", "alt") and (
            TR_RING == "sync" or _ring[0] % 2 == 0) else nc.scalar
        _ring[0] += 1
        eng.dma_start_transpose(out_ap, in_ap)

    def bc(scale_ap, nkb):
        # [128, nkb] f32 -> [128, nkb, 128] with stride-0 inner broadcast
        return scale_ap.unsqueeze(2).broadcast_to([P, nkb, P])

    # Split 1/448 so that s = RN(amax*c_hi + amax*c_lo) is exactly
    # RN(amax/448): amax is bf16-valued (8-bit mantissa) so amax*c_hi is
    # exact, and m/7 binary expansions have no long same-bit runs, so the
    # final rounding always agrees with true division.
    _c = np.float64(1.0) / np.float64(448.0)
    _m, _e = np.frexp(_c)
    C448_HI = float(np.float32(np.ldexp(np.floor(np.ldexp(_m, 16)), int(_e) - 16)))
    C448_LO = float(np.float32(_c - np.float64(C448_HI)))

    def quant_scales(pool, amax, nkb, tagp):
        """amax [128, nkb] f32 (abs-max) -> (rs, s2): rs = RNE(0.5/scale),
        s2 = 2*scale, scale = RNE(clip(amax)/448) exactly as the reference."""
        nc.vector.tensor_scalar_max(amax[:], amax[:], 1e-4)
        s = pool.tile([P, nkb], F32, tag=tagp + "_s")
        nc.vector.tensor_scalar_mul(s[:], amax[:], C448_LO)
        nc.vector.scalar_tensor_tensor(
            s[:], amax[:], C448_HI, s[:],
            op0=MUL, op1=mybir.AluOpType.add,
        )
        rs = pool.tile([P, nkb], F32, tag=tagp + "_rs")
        nc.vector.reciprocal(rs[:], s[:])
        nc.vector.tensor_scalar_mul(rs[:], rs[:], 0.5)
        s2 = pool.tile([P, nkb], F32, tag=tagp + "_s2")
        nc.vector.tensor_scalar_mul(s2[:], s[:], 2.0)
        return rs, s2

    with tile.TileContext(nc) as tc, tc.tile_pool(name="hT", bufs=1) as p_hT:
        hT = p_hT.tile([P, FB * NS], BF16)
        with (
            tc.tile_pool(name="xdT", bufs=1) as p_xdT,
            tc.tile_pool(name="scr", bufs=1, space="DRAM") as p_dram,
            tc.tile_pool(name="cw", bufs=1) as p_cw,
            tc.tile_pool(name="cs", bufs=2) as p_cs,
            tc.tile_pool(name="psA", bufs=4, space="PSUM") as p_psA,
        ):
            xdT = p_xdT.tile([P, KB * 512], BF16)
            w1dT = p_cw.tile([P, KB * 512], BF16)
            xscr = xscr_d
            w1scr = p_dram.tile([F, H], BF16)
            QW = QKB * P

            qpools = [None, None, None]

            def quant_rows(src_ap, dst_scr, rsl, pare, tagp):
                p_qw, p_qq, p_qsc = qpools
                """Quantize+dequantize one 128-row strip of src (natural
                layout) into the DRAM scratch, reproducing the reference
                fp8 grid.  pare=True adds the cross-partition block-max
                (weights); else scales are per-row (tokens)."""
                amax = p_qsc.tile([P, KB], F32, tag="amax")
                chunks = []
                for c in range(NCH):
                    qt = p_qw.tile([P, QW], BF16, tag="nt")
                    nc.sync.dma_start(
                        qt[:], src_ap[rsl, c * QW:(c + 1) * QW]
                    )
                    qt3 = qt[:].rearrange("p (k b) -> p k b", b=P)
                    nc.vector.tensor_reduce(
                        amax[:, c * QKB:(c + 1) * QKB], qt3, axis=X_AX,
                        op=mybir.AluOpType.max, apply_absolute_value=True,
                    )
                    chunks.append(qt3)
                if pare:
                    wam = p_qsc.tile([P, KB], F32, tag="wam")
                    nc.gpsimd.partition_all_reduce(
                        wam[:], amax[:], channels=P,
                        reduce_op=bass_isa.ReduceOp.absmax,
                    )
                    amax = wam
                rs, s2 = quant_scales(p_qsc, amax, KB, tagp)
                for c in range(NCH):
                    ksl = slice(c * QKB, (c + 1) * QKB)
                    q8 = p_qq.tile([P, QW], FP8, tag="q8")
                    q83 = q8[:].rearrange("p (k b) -> p k b", b=P)
                    nc.vector.tensor_tensor(
                        q83, chunks[c], bc(rs[:, ksl], QKB), op=MUL
                    )
                    dq = p_qq.tile([P, QW], BF16, tag="dq")
                    dq3 = dq[:].rearrange("p (k b) -> p k b", b=P)
                    nc.vector.tensor_tensor(
                        dq3, q83, bc(s2[:, ksl], QKB), op=MUL
                    )
                    nc.sync.dma_start(
                        dst_scr[rsl, c * QW:(c + 1) * QW], dq[:]
                    )

            with (
                tc.tile_pool(name="qw", bufs=NCH + 2) as p_qw,
                tc.tile_pool(name="qq", bufs=2) as p_qq,
                tc.tile_pool(name="qsc", bufs=2) as p_qsc,
            ):
                qpools[:] = [p_qw, p_qq, p_qsc]
                # phase X: quantize x rows, spill xd to scratch
                for st in range(ST):
                    quant_rows(
                        x_d.ap(), xscr.ap(),
                        slice(st * P, (st + 1) * P), False, "x"
                    )
                # phase W1: quantize w1 rows (128x128 blocks), spill w1d
                for fb in range(FB):
                    quant_rows(
                        w1_d.ap(), w1scr,
                        slice(fb * P, (fb + 1) * P), True, "w"
                    )

            # ---------------- passes: gemm1 + silu -> hT ---------------
            for p in range(NP):
                ssl = slice(p * 512, (p + 1) * 512)
                for kb in range(KB):
                    tr_dma(
                        xdT[:, kb * 512:(kb + 1) * 512],
                        xscr.ap()[ssl, kb * P:(kb + 1) * P],
                    )
                for fg in range(FB // FG):
                    for kb in range(KB):
                        tr_dma(
                            w1dT[:, kb * 512:(kb + 1) * 512],
                            w1scr[fg * FG * P:(fg + 1) * FG * P,
                                  kb * P:(kb + 1) * P],
                        )
                    for g in range(FG):
                        fb = fg * FG + g
                        ps = p_psA.tile([P, 512], F32, tag="ps")
                        for kb in range(KB):
                            nc.tensor.matmul(
                                ps[:],
                                w1dT[:, kb * 512 + g * P: kb * 512 + (g + 1) * P],
                                xdT[:, kb * 512:(kb + 1) * 512],
                                start=(kb == 0), stop=(kb == KB - 1),
                            )
                        o0b = p_cs.tile([P, 512], BF16, tag="o0b")
                        nc.scalar.copy(o0b[:], ps[:])
                        sg = p_cs.tile([P, 512], BF16, tag="sg")
                        nc.scalar.activation(
                            sg[:], o0b[:], mybir.ActivationFunctionType.Sigmoid
                        )
                        act = p_cs.tile([P, 512], BF16, tag="act")
                        nc.vector.tensor_mul(act[:], o0b[:], sg[:])
                        nc.vector.tensor_mul(
                            hT[:, fb * NS + p * 512: fb * NS + (p + 1) * 512],
                            act[:], o0b[:],
                        )

        # ---------------- phase D ----------------
        # (xdT/w1dT/scratch/w-pools released above; hT persists)
        with (
            tc.tile_pool(name="dw", bufs=2) as p_dw,
            tc.tile_pool(name="do", bufs=4) as p_do,
            tc.tile_pool(name="psB", bufs=4, space="PSUM") as p_psB,
        ):
            for sc in range(NSC):
                w2T = p_dw.tile([P, FB * SC], BF16, tag="w2T")
                for fb in range(FB):
                    nc.sync.dma_start_transpose(
                        w2T[:, fb * SC:(fb + 1) * SC],
                        w2_d.ap()[sc * SC:(sc + 1) * SC, fb * P:(fb + 1) * P],
                    )
                for hsub in range(SC // 512):
                    for st in range(ST):
                        ps2 = p_psB.tile([P, 512], F32, tag="ps2")
                        for fb in range(FB):
                            nc.tensor.matmul(
                                ps2[:],
                                hT[:, fb * NS + st * P: fb * NS + (st + 1) * P],
                                w2T[:, fb * SC + hsub * 512: fb * SC + (hsub + 1) * 512],
                                start=(fb == 0), stop=(fb == FB - 1),
                            )
                        ob = p_do.tile([P, 512], BF16, tag="ob")
                        if st % 2 == 0:
                            nc.vector.tensor_copy(ob[:], ps2[:])
                        else:
                            nc.scalar.copy(ob[:], ps2[:])
                        nc.sync.dma_start(
                            out_d.ap()[st * P:(st + 1) * P,
                                       sc * SC + hsub * 512: sc * SC + (hsub + 1) * 512],
                            ob[:],
                        )

    nc.compile()
    return nc


_PROG_CACHE = {}


def _get_program(NS, H, F, num_devices=8):
    key = (NS, H, F, num_devices)
    if key not in _PROG_CACHE:
        _PROG_CACHE[key] = build_program(NS, H, F, num_devices)
    return _PROG_CACHE[key]


NCORES = 8


def kernel(x, w1, w2, w3=None, **_ignored):
    """Full-input entry point: shards tokens across 8 NeuronCores."""
    from concourse.bass_utils import run_bass_kernel_spmd

    x = np.asarray(x)
    w1 = np.asarray(w1)
    w2 = np.asarray(w2)
    S, H = x.shape
    F = w1.shape[0]
    NS = S // NCORES
    nc = _get_program(NS, H, F, NCORES)
    in_maps = [
        {
            "x": np.ascontiguousarray(x[i * NS:(i + 1) * NS]),
            "w1": w1,
            "w2": w2,
        }
        for i in range(NCORES)
    ]
    res = run_bass_kernel_spmd(nc, in_maps, core_ids=list(range(NCORES)))
    return np.concatenate(
        [res.results[i]["out"] for i in range(NCORES)], axis=0
    )
